# revision 1
# baseline (speedup 1.0000x reference)
"""Packed causal GQA attention (B=4 x S=1024, H=32, KVH=8, D=DV=128, fp32)
for 8 Trainium2 NeuronCores.

Sharding: tensor-parallel over KV heads. Core c owns kv head c and its
GQA group of 4 query heads (4c..4c+3). No cross-core communication.

Per-core algorithm, per (batch b, head h):
  - Q^T, K^T tiles [d, t] in fp16 built via SWDGE cast-load (fp32->fp16)
    followed by HWDGE DMA-transpose (SBUF->SBUF). fp16 round-off is ~2.4e-4
    relative, matching the fp32r matmul precision used downstream.
  - S^T[k, q] = K^T.T @ Q^T per 128-row k-block (PE, fp16 inputs, fp32 PSUM),
    only causal (k <= q) column ranges are computed.
  - P^T = Exp(SCALE * S^T) on the scalar engine, written as float32r tiles;
    the strictly-upper triangle of each diagonal block is zeroed with a
    gpsimd affine_select.
  - out^T[dv, q] = sum_kb V[kb].T @ P^T[kb]  and  l[q] = sum_kb 1.T @ P^T[kb]
    (fp32r matmuls accumulating in PSUM; the ones-matmul broadcasts the
    softmax denominator across all 128 partitions).
  - out = out^T * (1/l) via DVE reciprocal_approx_fast + multiply, stored
    to DRAM in [dv, q] layout; the host transposes back when unsharding.

NOTE: plain DMAs must stay on SWDGE (nc.gpsimd) — concurrent HWDGE plain
copies corrupt in-flight HWDGE DMA-transposes (xbar mode conflict).
"""

import numpy as np

import concourse.bacc as bacc
import concourse.tile as tile
from concourse import mybir, bass_utils

T = 4096          # packed tokens
SEQ = 1024        # per-sequence length
B = T // SEQ      # 4 sequences
H = 32            # query heads (total)
KVH = 8           # kv heads (total)
D = 128           # head size
DV = 128          # value head size
NCORES = 8
HPC = H // NCORES         # 4 query heads per core
NB = SEQ // 128           # 8 k-blocks per sequence
SCALE = 0.08838834764831845

F16 = mybir.dt.float16
F32 = mybir.dt.float32
F32R = mybir.dt.float32r

_BUILD_CACHE = {}


def _build_nc():
    nc = bacc.Bacc("TRN2", target_bir_lowering=False, debug=False,
                   num_devices=NCORES)
    q_dram = nc.dram_tensor("q", [T, HPC * D], F32, kind="ExternalInput").ap()
    k_dram = nc.dram_tensor("k", [T, D], F32, kind="ExternalInput").ap()
    v_dram = nc.dram_tensor("v", [T, DV], F32R, kind="ExternalInput").ap()
    ones_dram = nc.dram_tensor("ones", [128, 128], F32R, kind="ExternalInput").ap()
    # out_t[b*HPC + h, dv, q]  (transposed per-head output; host untransposes)
    out_dram = nc.dram_tensor("out_t", [B * HPC, DV, SEQ], F32,
                              kind="ExternalOutput").ap()

    with tile.TileContext(nc) as tc:
        with tc.tile_pool(name="consts", bufs=1) as consts, \
             tc.tile_pool(name="kv", bufs=1) as kv_pool, \
             tc.tile_pool(name="stage", bufs=2) as stage, \
             tc.tile_pool(name="qt", bufs=4) as qt_pool, \
             tc.tile_pool(name="pt", bufs=2) as pt_pool, \
             tc.tile_pool(name="work", bufs=2) as work, \
             tc.tile_pool(name="pp_s", bufs=3, space="PSUM") as pp_s, \
             tc.tile_pool(name="pp_o", bufs=2, space="PSUM") as pp_o, \
             tc.tile_pool(name="pp_l", bufs=2, space="PSUM") as pp_l:

            ones_sb = consts.tile([128, 128], F32R, tag="ones")
            nc.gpsimd.dma_start(ones_sb[:], ones_dram[:])

            for b in range(B):
                rows = slice(b * SEQ, (b + 1) * SEQ)

                # K^T [d, t] fp16 for this sequence
                k_stage = stage.tile([128, NB, D], F16, tag="kst")
                nc.gpsimd.dma_start(
                    k_stage[:], k_dram[rows, :].rearrange("(nb p) d -> p nb d", p=128))
                kt = kv_pool.tile([128, SEQ], F16, tag=f"kt{b}")
                for kb in range(NB):
                    nc.sync.dma_start(kt[:, kb * 128:(kb + 1) * 128],
                                      k_stage[:, kb, :], transpose=True)

                # V [k, dv] natural layout, fp32r
                v_sb = kv_pool.tile([128, NB, DV], F32R, tag=f"v{b}")
                nc.gpsimd.dma_start(
                    v_sb[:], v_dram[rows, :].rearrange("(nb p) d -> p nb d", p=128))

                # Q staging for all 4 heads of this sequence
                q_stage = stage.tile([128, NB, HPC * D], F16, tag="qst")
                nc.gpsimd.dma_start(
                    q_stage[:], q_dram[rows, :].rearrange("(nb p) hd -> p nb hd", p=128))

                for h in range(HPC):
                    qt = qt_pool.tile([128, SEQ], F16, tag="qt")
                    for qb in range(NB):
                        nc.sync.dma_start(qt[:, qb * 128:(qb + 1) * 128],
                                          q_stage[:, qb, h * D:(h + 1) * D],
                                          transpose=True)

                    # ---- scores + exp per k-block (S^T layout) ----
                    pts = []
                    for kb in range(NB):
                        pt = pt_pool.tile([128, SEQ - 128 * kb], F32R, tag=f"pt{kb}")
                        for qc in range(kb // 4, 2):
                            qs = max(128 * kb, 512 * qc)
                            qe = 512 * (qc + 1)
                            ncols = qe - qs
                            ps = pp_s.tile([128, 512], F32, tag="ps_s")
                            nc.tensor.matmul(
                                ps[:, :ncols],
                                kt[:, kb * 128:(kb + 1) * 128],
                                qt[:, qs:qe],
                                start=True, stop=True, skip_group_check=True)
                            nc.scalar.activation(
                                pt[:, qs - 128 * kb:qe - 128 * kb], ps[:, :ncols],
                                mybir.ActivationFunctionType.Exp, scale=SCALE)
                        # zero strictly-upper triangle of the diagonal block
                        nc.gpsimd.affine_select(
                            out=pt[:, 0:128], in_=pt[:, 0:128],
                            compare_op=mybir.AluOpType.is_ge,
                            fill=0.0, base=0,
                            pattern=[[1, 128]], channel_multiplier=-1)
                        pts.append(pt)

                    # ---- PV + denominator, then normalize ----
                    out_sb = work.tile([128, SEQ], F32, tag="out_sb")
                    for qc in range(2):
                        kbs = list(range(0, 4 * qc + 4))
                        ps_o = pp_o.tile([128, 512], F32, tag="ps_o")
                        ps_l = pp_l.tile([128, 512], F32, tag="ps_l")
                        for kb in kbs:
                            qs = max(128 * kb, 512 * qc)
                            qe = 512 * (qc + 1)
                            rhs = pts[kb][:, qs - 128 * kb:qe - 128 * kb]
                            flags = dict(start=(kb == 0), stop=(kb == kbs[-1]),
                                         skip_group_check=True)
                            nc.tensor.matmul(
                                ps_o[:, qs - 512 * qc:512], v_sb[:, kb, :], rhs, **flags)
                            nc.tensor.matmul(
                                ps_l[:, qs - 512 * qc:512], ones_sb[:], rhs, **flags)
                        rsb = work.tile([128, 512], F32, tag="rsb")
                        nc.vector.reciprocal_approx_fast(rsb[:], ps_l[:])
                        nc.vector.tensor_tensor(
                            out=out_sb[:, qc * 512:(qc + 1) * 512],
                            in0=ps_o[:], in1=rsb[:], op=mybir.AluOpType.mult)

                    nc.gpsimd.dma_start(out_dram[b * HPC + h], out_sb[:])

    nc.compile()
    return nc


def run_sharded(query, key, value, trace=False):
    """Shard over 8 cores, run the bass kernel, unshard. Returns
    (out [T, H*DV] fp32, BassKernelResults)."""
    query = np.ascontiguousarray(np.asarray(query, dtype=np.float32))
    key = np.ascontiguousarray(np.asarray(key, dtype=np.float32))
    value = np.ascontiguousarray(np.asarray(value, dtype=np.float32))

    if "nc" not in _BUILD_CACHE:
        _BUILD_CACHE["nc"] = _build_nc()
    nc = _BUILD_CACHE["nc"]

    ones = np.ones((128, 128), np.float32)
    in_maps = []
    for c in range(NCORES):
        in_maps.append({
            "q": np.ascontiguousarray(query[:, c * HPC * D:(c + 1) * HPC * D]),
            "k": np.ascontiguousarray(key[:, c * D:(c + 1) * D]),
            "v": np.ascontiguousarray(value[:, c * DV:(c + 1) * DV]),
            "ones": ones,
        })

    res = bass_utils.run_bass_kernel_spmd(
        nc, in_maps, core_ids=list(range(NCORES)), trace=trace)

    outs = []
    for c in range(NCORES):
        ot = res.results[c]["out_t"]                # [B*HPC, DV, SEQ]
        o = ot.reshape(B, HPC, DV, SEQ).transpose(0, 3, 1, 2).reshape(T, HPC * DV)
        outs.append(o)
    return np.concatenate(outs, axis=1), res


def kernel(query, key, value, seq_len=1024, **_unused):
    assert int(seq_len) == SEQ, f"kernel hardcodes seq_len={SEQ}, got {seq_len}"
    out, _ = run_sharded(query, key, value, trace=False)
    return out


# revision 2
# speedup vs baseline: 2.0689x; 2.0689x over previous
"""Packed causal GQA attention (B=4 x S=1024, H=32, KVH=8, D=DV=128, fp32)
for 8 Trainium2 NeuronCores.

Sharding: tensor-parallel over KV heads. Core c owns kv head c and its
GQA group of 4 query heads (4c..4c+3). No cross-core communication.

Per-core pipeline (all (b, h) = 4 sequences x 4 heads):
  - Q^T/K^T [d, t] fp16 tiles built once per core: SWDGE cast-load
    (fp32->fp16) into [t%128, t//128, d] staging, then ONE batched HWDGE
    DMA-transpose per head / per K ([128, 4096] -> [128, 32, 128]).
  - Per (b,h,kb): S^T[k, q] = K^T.T @ Q^T on PE (fp16, fp32 PSUM), causal
    column ranges only; P^T = Exp(SCALE*S^T) on ACT -> fp16 tiles; diagonal
    block upper triangle zeroed by gpsimd affine_select.
  - out^T[dv, q] = sum_kb V[kb].T @ P^T[kb], l[q] = sum_kb 1.T @ P^T[kb]
    (fp16 matmuls, fp32 PSUM accumulation; the ones-matmul broadcasts the
    softmax denominator over all 128 partitions).
  - out = out^T * (1/l) via DVE reciprocal_approx_fast + multiply; stored
    [dv, q]; host untransposes during unshard.

NOTE: plain DMAs stay on SWDGE (nc.gpsimd) — concurrent HWDGE plain copies
corrupt in-flight HWDGE DMA-transposes (xbar mode conflict).
"""

import numpy as np

import concourse.bacc as bacc
import concourse.tile as tile
from concourse import mybir, bass_utils

T = 4096          # packed tokens
SEQ = 1024        # per-sequence length
B = T // SEQ      # 4 sequences
H = 32            # query heads (total)
KVH = 8           # kv heads (total)
D = 128           # head size
DV = 128          # value head size
NCORES = 8
HPC = H // NCORES         # 4 query heads per core
NB = SEQ // 128           # 8 k-blocks per sequence
NBT = T // 128            # 32 token blocks total
SCALE = 0.08838834764831845

F16 = mybir.dt.float16
F32 = mybir.dt.float32

_BUILD_CACHE = {}


def _build_nc():
    nc = bacc.Bacc("TRN2", target_bir_lowering=False, debug=False,
                   num_devices=NCORES)
    q_dram = nc.dram_tensor("q", [T, HPC * D], F32, kind="ExternalInput").ap()
    k_dram = nc.dram_tensor("k", [T, D], F32, kind="ExternalInput").ap()
    v_dram = nc.dram_tensor("v", [T, DV], F32, kind="ExternalInput").ap()
    # out_t[b*HPC + h, dv, q]  (transposed per-head output; host untransposes)
    out_dram = nc.dram_tensor("out_t", [B * HPC, DV, SEQ], F32,
                              kind="ExternalOutput").ap()

    with tile.TileContext(nc) as tc:
        with tc.tile_pool(name="consts", bufs=1) as consts, \
             tc.tile_pool(name="persist", bufs=1) as persist, \
             tc.tile_pool(name="stage", bufs=2) as stage, \
             tc.tile_pool(name="pt", bufs=2) as pt_pool, \
             tc.tile_pool(name="work", bufs=2) as work, \
             tc.tile_pool(name="pp_s", bufs=4, space="PSUM") as pp_s, \
             tc.tile_pool(name="pp_o", bufs=2, space="PSUM") as pp_o, \
             tc.tile_pool(name="pp_l", bufs=2, space="PSUM") as pp_l:

            ones_sb = consts.tile([128, 128], F16, tag="ones")
            nc.vector.memset(ones_sb[:], 1.0)

            # ---- K^T for all sequences: one cast-load + one batched transpose
            k_stage = stage.tile([128, NBT, D], F16, tag="kst")
            nc.gpsimd.dma_start(
                k_stage[:], k_dram.rearrange("(nb p) d -> p nb d", p=128))
            kt = persist.tile([128, NBT, 128], F16, tag="kt")
            nc.sync.dma_start(kt[:], k_stage[:], transpose=True)

            # ---- V natural layout fp16 [k%128, kblock, dv]
            v_sb = persist.tile([128, NBT, DV], F16, tag="v")
            nc.gpsimd.dma_start(
                v_sb[:], v_dram.rearrange("(nb p) d -> p nb d", p=128))

            # ---- Q^T per head: cast-load + one batched transpose each
            qts = []
            for h in range(HPC):
                q_stage = stage.tile([128, NBT, D], F16, tag="qst")
                nc.gpsimd.dma_start(
                    q_stage[:],
                    q_dram[:, h * D:(h + 1) * D].rearrange(
                        "(nb p) d -> p nb d", p=128))
                qt = persist.tile([128, NBT, 128], F16, tag=f"qt{h}")
                nc.sync.dma_start(qt[:], q_stage[:], transpose=True)
                qts.append(qt)

            for b in range(B):
                for h in range(HPC):
                    qt = qts[h]

                    # ---- scores + exp per k-block (S^T layout) ----
                    pts = []
                    for kb in range(NB):
                        pt = pt_pool.tile([128, SEQ - 128 * kb], F16, tag=f"pt{kb}")
                        for qc in range(kb // 4, 2):
                            qs = max(128 * kb, 512 * qc)
                            qe = 512 * (qc + 1)
                            ncols = qe - qs
                            ps = pp_s.tile([128, 512], F32, tag="ps_s")
                            nc.tensor.matmul(
                                ps[:, :ncols],
                                kt[:, b * NB + kb, :],
                                qt[:, b * NB + qs // 128:b * NB + qe // 128, :],
                                start=True, stop=True, skip_group_check=True)
                            nc.scalar.activation(
                                pt[:, qs - 128 * kb:qe - 128 * kb], ps[:, :ncols],
                                mybir.ActivationFunctionType.Exp, scale=SCALE)
                        # zero strictly-upper triangle of the diagonal block
                        nc.gpsimd.affine_select(
                            out=pt[:, 0:128], in_=pt[:, 0:128],
                            compare_op=mybir.AluOpType.is_ge,
                            fill=0.0, base=0,
                            pattern=[[1, 128]], channel_multiplier=-1)
                        pts.append(pt)

                    # ---- PV + denominator, then normalize ----
                    out_sb = work.tile([128, SEQ], F32, tag="out_sb")
                    for qc in range(2):
                        kbs = list(range(0, 4 * qc + 4))
                        ps_o = pp_o.tile([128, 512], F32, tag="ps_o")
                        ps_l = pp_l.tile([128, 512], F32, tag="ps_l")
                        for kb in kbs:
                            qs = max(128 * kb, 512 * qc)
                            qe = 512 * (qc + 1)
                            rhs = pts[kb][:, qs - 128 * kb:qe - 128 * kb]
                            flags = dict(start=(kb == 0), stop=(kb == kbs[-1]),
                                         skip_group_check=True)
                            nc.tensor.matmul(
                                ps_o[:, qs - 512 * qc:512],
                                v_sb[:, b * NB + kb, :], rhs, **flags)
                            nc.tensor.matmul(
                                ps_l[:, qs - 512 * qc:512], ones_sb[:], rhs, **flags)
                        rsb = work.tile([128, 512], F32, tag="rsb")
                        nc.vector.reciprocal_approx_fast(rsb[:], ps_l[:])
                        nc.vector.tensor_tensor(
                            out=out_sb[:, qc * 512:(qc + 1) * 512],
                            in0=ps_o[:], in1=rsb[:], op=mybir.AluOpType.mult)

                    nc.gpsimd.dma_start(out_dram[b * HPC + h], out_sb[:])

    nc.compile()
    return nc


def run_sharded(query, key, value, trace=False):
    """Shard over 8 cores, run the bass kernel, unshard. Returns
    (out [T, H*DV] fp32, BassKernelResults)."""
    query = np.ascontiguousarray(np.asarray(query, dtype=np.float32))
    key = np.ascontiguousarray(np.asarray(key, dtype=np.float32))
    value = np.ascontiguousarray(np.asarray(value, dtype=np.float32))

    if "nc" not in _BUILD_CACHE:
        _BUILD_CACHE["nc"] = _build_nc()
    nc = _BUILD_CACHE["nc"]

    in_maps = []
    for c in range(NCORES):
        in_maps.append({
            "q": np.ascontiguousarray(query[:, c * HPC * D:(c + 1) * HPC * D]),
            "k": np.ascontiguousarray(key[:, c * D:(c + 1) * D]),
            "v": np.ascontiguousarray(value[:, c * DV:(c + 1) * DV]),
        })

    res = bass_utils.run_bass_kernel_spmd(
        nc, in_maps, core_ids=list(range(NCORES)), trace=trace)

    outs = []
    for c in range(NCORES):
        ot = res.results[c]["out_t"]                # [B*HPC, DV, SEQ]
        o = ot.reshape(B, HPC, DV, SEQ).transpose(0, 3, 1, 2).reshape(T, HPC * DV)
        outs.append(o)
    return np.concatenate(outs, axis=1), res


def kernel(query, key, value, seq_len=1024, **_unused):
    assert int(seq_len) == SEQ, f"kernel hardcodes seq_len={SEQ}, got {seq_len}"
    out, _ = run_sharded(query, key, value, trace=False)
    return out
